# revision 5
# baseline (speedup 1.0000x reference)
"""Trainium2 Bass kernel for nn_ControllerCell (controller+plant MLP cell).

Contract: kernel(**inputs) takes FULL unsharded inputs (numpy, float32) and
returns the FULL output tuple matching the reference:
    (a4, citdl', crtdl', cotdl', pitdl', potdl')

Strategy:
  - Data-parallel over the batch (column) dim B=16384 across 8 NeuronCores
    (2048 columns per core); tiny weight matrices replicated.
  - The TDL shift updates are pure row shifts of the inputs -> assembled on
    the host. Only the two computed rows (a2 = controller output, a4 = plant
    output) require device compute.
  - Device per core computes:
        n1 = Wc @ [citdl;crtdl;cotdl] (+cb1 via ACT bias), a1 = tanh(n1)
        a2 = clw @ a1                  (cb2 folded on host)
        n3 = Wp' @ [a2;pitdl;potdl] (+pb1' via ACT bias), a3 = tanh(n3)
        a4 = plw @ a3                  (pb2 folded on host)
    with Wp' the plant weight matrix reordered so the a2 column is K-row 0,
    and pb1' = pb1 + piw[:,63]*cb2 (so the device a2 needs no bias).
  - Matmuls run in float32r (full-rate fp32 on the PE array).
"""

import numpy as np

N_CORES = 8
B = 16384
BC = B // N_CORES  # 2048 columns per core
H = 1024
NM = H // 128      # 8 M-tiles of 128 rows

_CACHE = {}


def _build_nc():
    import concourse.mybir as mybir
    import concourse.tile as tile
    from concourse import bacc
    from contextlib import ExitStack

    F32 = mybir.dt.float32
    F32R = mybir.dt.float32r
    AF = mybir.ActivationFunctionType

    nc = bacc.Bacc()

    xc_d = nc.declare_dram_parameter("xc", [192, BC], F32R, isOutput=False)
    xp_d = nc.declare_dram_parameter("xp", [127, BC], F32R, isOutput=False)
    wct_d = nc.declare_dram_parameter("wct", [192, H], F32R, isOutput=False)
    wpt_d = nc.declare_dram_parameter("wpt", [128, H], F32R, isOutput=False)
    clw_d = nc.declare_dram_parameter("clw8", [128, NM], F32R, isOutput=False)
    plw_d = nc.declare_dram_parameter("plw8", [128, NM], F32R, isOutput=False)
    cb1_d = nc.declare_dram_parameter("cb18", [128, NM], F32, isOutput=False)
    pb1_d = nc.declare_dram_parameter("pb18", [128, NM], F32, isOutput=False)
    a2_d = nc.declare_dram_parameter("a2o", [1, BC], F32R, isOutput=True)
    a4_d = nc.declare_dram_parameter("a4o", [1, BC], F32, isOutput=True)

    with ExitStack() as ctx:
        tc = ctx.enter_context(tile.TileContext(nc))
        const = ctx.enter_context(tc.tile_pool(name="const", bufs=1))
        acts = ctx.enter_context(tc.tile_pool(name="acts", bufs=3))
        psum = ctx.enter_context(tc.tile_pool(name="psum", bufs=2, space="PSUM"))
        psump = ctx.enter_context(tc.tile_pool(name="psump", bufs=1, space="PSUM"))

        # --- load inputs/weights to SBUF ---
        xc0 = const.tile([128, BC], F32R)
        nc.sync.dma_start(out=xc0, in_=xc_d[0:128, :])
        xc1 = const.tile([64, BC], F32R)
        nc.sync.dma_start(out=xc1, in_=xc_d[128:192, :])
        xpt = const.tile([128, BC], F32R)
        nc.sync.dma_start(out=xpt[1:128, :], in_=xp_d[:, :])  # row 0 = a2, later
        wct0 = const.tile([128, H], F32R)
        nc.sync.dma_start(out=wct0, in_=wct_d[0:128, :])
        wct1 = const.tile([64, H], F32R)
        nc.sync.dma_start(out=wct1, in_=wct_d[128:192, :])
        wpt = const.tile([128, H], F32R)
        nc.sync.dma_start(out=wpt, in_=wpt_d[:, :])
        clw8 = const.tile([128, NM], F32R)
        nc.sync.dma_start(out=clw8, in_=clw_d[:, :])
        plw8 = const.tile([128, NM], F32R)
        nc.sync.dma_start(out=plw8, in_=plw_d[:, :])
        cb18 = const.tile([128, NM], F32)
        nc.sync.dma_start(out=cb18, in_=cb1_d[:, :])
        pb18 = const.tile([128, NM], F32)
        nc.sync.dma_start(out=pb18, in_=pb1_d[:, :])

        a2ps = psump.tile([1, BC], F32, tag="proj")

        # --- layer 1: controller ---
        for m in range(NM):
            ms = slice(m * 128, (m + 1) * 128)
            for nh in range(2):
                n1 = psum.tile([128, 1024], F32, tag="n1")
                for ch in range(2):
                    c0 = nh * 1024 + ch * 512
                    cs = slice(c0, c0 + 512)
                    ps = slice(ch * 512, (ch + 1) * 512)
                    nc.tensor.matmul(
                        n1[:, ps],
                        lhsT=wct0[:, ms],
                        rhs=xc0[:, cs],
                        start=True,
                        stop=False,
                    )
                    nc.tensor.matmul(
                        n1[:, ps],
                        lhsT=wct1[:, ms],
                        rhs=xc1[:, cs],
                        start=False,
                        stop=True,
                    )
                a1 = acts.tile([128, 1024], F32R, tag="a1")
                nc.scalar.activation(a1, n1, AF.Tanh, bias=cb18[:, m : m + 1])
                for ch in range(2):
                    c0 = nh * 1024 + ch * 512
                    cs = slice(c0, c0 + 512)
                    nc.tensor.matmul(
                        a2ps[0:1, cs],
                        lhsT=clw8[:, m : m + 1],
                        rhs=a1[:, ch * 512 : (ch + 1) * 512],
                        start=(m == 0),
                        stop=(m == NM - 1),
                    )

        # a2 -> row 0 of plant rhs (split across DVE+ACT to halve latency)
        nc.vector.tensor_copy(xpt[0:1, 0:1024], a2ps[0:1, 0:1024])
        nc.scalar.activation(xpt[0:1, 1024:2048], a2ps[0:1, 1024:2048], AF.Copy)
        nc.sync.dma_start(out=a2_d[:, :], in_=xpt[0:1, :])

        a4ps = psump.tile([1, BC], F32, tag="proj")

        # --- layer 2: plant ---
        for m in range(NM):
            ms = slice(m * 128, (m + 1) * 128)
            for nh in range(2):
                n3 = psum.tile([128, 1024], F32, tag="n1")
                for ch in range(2):
                    c0 = nh * 1024 + ch * 512
                    cs = slice(c0, c0 + 512)
                    ps = slice(ch * 512, (ch + 1) * 512)
                    nc.tensor.matmul(
                        n3[:, ps],
                        lhsT=wpt[:, ms],
                        rhs=xpt[:, cs],
                        start=True,
                        stop=True,
                    )
                a3 = acts.tile([128, 1024], F32R, tag="a1")
                nc.scalar.activation(a3, n3, AF.Tanh, bias=pb18[:, m : m + 1])
                for ch in range(2):
                    c0 = nh * 1024 + ch * 512
                    cs = slice(c0, c0 + 512)
                    nc.tensor.matmul(
                        a4ps[0:1, cs],
                        lhsT=plw8[:, m : m + 1],
                        rhs=a3[:, ch * 512 : (ch + 1) * 512],
                        start=(m == 0),
                        stop=(m == NM - 1),
                    )

        a4sb = const.tile([1, BC], F32)
        nc.vector.tensor_copy(a4sb[0:1, 0:1024], a4ps[0:1, 0:1024])
        nc.scalar.activation(a4sb[0:1, 1024:2048], a4ps[0:1, 1024:2048], AF.Copy)
        nc.sync.dma_start(out=a4_d[:, :], in_=a4sb[0:1, :])

    nc.finalize()
    return nc


def _get_nc():
    if "nc" not in _CACHE:
        _CACHE["nc"] = _build_nc()
    return _CACHE["nc"]


def run(inputs, trace=False):
    """Run the device kernel. Returns (outputs_tuple, exec_time_ns|None)."""
    from concourse.bass_utils import run_bass_kernel_spmd

    f32 = np.float32
    g = {k: np.asarray(v, dtype=f32) for k, v in inputs.items()}

    # Host-side packing (tiny arrays; negligible cost vs device work).
    xc = np.ascontiguousarray(
        np.concatenate([g["citdl"], g["crtdl"], g["cotdl"]], axis=0)
    )  # [192, B]
    xp = np.ascontiguousarray(
        np.concatenate([g["pitdl"], g["potdl"]], axis=0)
    )  # [127, B]
    wct = np.ascontiguousarray(
        np.concatenate([g["ciw"], g["crw"], g["cow"]], axis=1).T
    )  # [192, H]
    # plant: K-row 0 = a2 column (piw[:,63]), rows 1..63 = pitdl cols,
    # rows 64..127 = potdl cols
    wpt = np.ascontiguousarray(
        np.concatenate([g["piw"][:, 63:64], g["piw"][:, :63], g["p_ow"]], axis=1).T
    )  # [128, H]
    clw8 = np.ascontiguousarray(g["clw"][0].reshape(NM, 128).T)  # [128, 8]
    plw8 = np.ascontiguousarray(g["plw"][0].reshape(NM, 128).T)
    cb18 = np.ascontiguousarray(g["cb1"][:, 0].reshape(NM, 128).T)
    pb1p = g["pb1"] + g["piw"][:, 63:64] * g["cb2"][0, 0]
    pb18 = np.ascontiguousarray(pb1p[:, 0].reshape(NM, 128).T)

    in_maps = []
    for c in range(N_CORES):
        cs = slice(c * BC, (c + 1) * BC)
        in_maps.append(
            {
                "xc": np.ascontiguousarray(xc[:, cs]),
                "xp": np.ascontiguousarray(xp[:, cs]),
                "wct": wct,
                "wpt": wpt,
                "clw8": clw8,
                "plw8": plw8,
                "cb18": cb18,
                "pb18": pb18,
            }
        )

    nc = _get_nc()
    res = run_bass_kernel_spmd(nc, in_maps, list(range(N_CORES)), trace=trace)

    a2 = np.concatenate([res.results[c]["a2o"] for c in range(N_CORES)], axis=1)
    a4 = np.concatenate([res.results[c]["a4o"] for c in range(N_CORES)], axis=1)
    a2 = (a2 + g["cb2"][0, 0]).astype(f32)  # [1, B] controller output
    a4 = (a4 + g["pb2"][0, 0]).astype(f32)  # [1, B] plant output

    out = (
        a4,
        np.concatenate([a2, g["citdl"][:-1]], axis=0),
        np.concatenate([g["reference"], g["crtdl"][:-1]], axis=0),
        np.concatenate([a4, g["cotdl"][:-1]], axis=0),
        np.concatenate([a2, g["pitdl"][:-1]], axis=0),
        np.concatenate([a4, g["potdl"][:-1]], axis=0),
    )
    return out, res.exec_time_ns


def kernel(**inputs):
    return run(inputs, trace=False)[0]


# revision 8
# speedup vs baseline: 1.4120x; 1.4120x over previous
"""Trainium2 Bass kernel for nn_ControllerCell (controller+plant MLP cell).

Contract: kernel(**inputs) takes FULL unsharded inputs (numpy, float32) and
returns the FULL output tuple matching the reference:
    (a4, citdl', crtdl', cotdl', pitdl', potdl')

Strategy:
  - Data-parallel over the batch (column) dim B=16384 across 8 NeuronCores
    (2048 columns per core); tiny weight matrices replicated.
  - TDL shift updates are pure row shifts of the inputs -> assembled on the
    host. Only the computed rows (a2 = controller output, a4 = plant output)
    require device compute.
  - Device math per core (bias terms folded on host where possible):
        n1 = Wc @ [citdl;crtdl;cotdl] (+cb1 via ACT bias), a1 = tanh(n1)
        a2 = clw @ a1                  (cb2 added on host)
        n3 = Wp' @ perm[a2;pitdl;potdl] (+pb1' via ACT bias), a3 = tanh(n3)
        a4 = plw @ a3                  (pb2 added on host)
  - Projections (clw@a1, plw@a3) use 4-way PE column-tiling: column chunk c
    writes its [1,512] result to PSUM partition 32c, all four concurrently.
  - The plant rhs uses a per-chunk ROTATED K-order (rotate by 32c) so that
    chunk c's a2 row sits at partition 32c -- exactly where the projection
    left it (engine copies are partition-preserving). The plant weights are
    pre-rotated per chunk on the host to match.
  - MM1's K=192 runs as a K=128 pass plus K=64 passes packed pairwise into
    PE row groups 0-63 / 64-127 (cotdl + its weights duplicated into the
    upper partitions) so two column chunks proceed concurrently.
  - All matmuls in float32r (full-rate fp32 on the PE array).
"""

import numpy as np

N_CORES = 8
B = 16384
BC = B // N_CORES  # 2048 columns per core
H = 1024
NM = H // 128      # 8 M-tiles of 128 rows
NC4 = 4            # column chunks of 512 per core

_CACHE = {}


def _build_nc():
    import concourse.mybir as mybir
    import concourse.tile as tile
    from concourse import bacc
    from contextlib import ExitStack

    F32 = mybir.dt.float32
    F32R = mybir.dt.float32r
    BF16 = mybir.dt.bfloat16
    AF = mybir.ActivationFunctionType

    nc = bacc.Bacc()

    # xc: [citdl; crtdl] (128 rows). xc1d: cotdl duplicated (rows 0-63 and
    # 64-127 identical). xp4: per-chunk rotated [a2-slot; pitdl; potdl].
    # wp4: per-chunk rotated plant weightsT, stacked [4*128, H].
    xc_d = nc.declare_dram_parameter("xc", [128, BC], F32R, isOutput=False)
    xc1_d = nc.declare_dram_parameter("xc1d", [128, BC], F32R, isOutput=False)
    xp_d = nc.declare_dram_parameter("xp4", [128, BC], F32R, isOutput=False)
    wct0_d = nc.declare_dram_parameter("wct0", [128, H], F32R, isOutput=False)
    wct1_d = nc.declare_dram_parameter("wct1d", [128, H], F32R, isOutput=False)
    wp4_d = nc.declare_dram_parameter("wp4", [4 * 128, H], F32R, isOutput=False)
    clw_d = nc.declare_dram_parameter("clw8", [128, NM], BF16, isOutput=False)
    plw_d = nc.declare_dram_parameter("plw8", [128, NM], BF16, isOutput=False)
    cb1_d = nc.declare_dram_parameter("cb18", [128, NM], F32, isOutput=False)
    pb1_d = nc.declare_dram_parameter("pb18", [128, NM], F32, isOutput=False)
    a2_d = nc.declare_dram_parameter("a2o", [1, BC], F32R, isOutput=True)
    a4_d = nc.declare_dram_parameter("a4o", [1, BC], F32, isOutput=True)

    with ExitStack() as ctx:
        tc = ctx.enter_context(tile.TileContext(nc))
        const = ctx.enter_context(tc.tile_pool(name="const", bufs=1))
        acts = ctx.enter_context(tc.tile_pool(name="acts", bufs=3))
        psum = ctx.enter_context(tc.tile_pool(name="psum", bufs=2, space="PSUM"))
        psump = ctx.enter_context(tc.tile_pool(name="psump", bufs=1, space="PSUM"))

        # --- loads; sync + scalar drive the two HWDGE rings ---
        wct0 = const.tile([128, H], F32R)
        nc.sync.dma_start(out=wct0, in_=wct0_d[:, :])
        xc0 = const.tile([128, BC], F32R)
        xc1d = const.tile([128, BC], F32R)
        wct1d = const.tile([128, H], F32R)
        nc.sync.dma_start(out=wct1d, in_=wct1_d[:, :])
        for c in range(NC4):
            cs = slice(512 * c, 512 * (c + 1))
            nc.sync.dma_start(out=xc0[:, cs], in_=xc_d[:, cs])
            nc.sync.dma_start(out=xc1d[:, cs], in_=xc1_d[:, cs])

        clw8 = const.tile([128, NM], BF16)
        nc.scalar.dma_start(out=clw8, in_=clw_d[:, :])
        cb18 = const.tile([128, NM], F32)
        nc.scalar.dma_start(out=cb18, in_=cb1_d[:, :])
        plw8 = const.tile([128, NM], BF16)
        nc.scalar.dma_start(out=plw8, in_=plw_d[:, :])
        pb18 = const.tile([128, NM], F32)
        nc.scalar.dma_start(out=pb18, in_=pb1_d[:, :])
        xp4 = const.tile([128, BC], F32R)
        wp4 = const.tile([128, 4 * H], F32R)
        for c in range(NC4):
            nc.scalar.dma_start(
                out=xp4[:, 512 * c : 512 * (c + 1)],
                in_=xp_d[:, 512 * c : 512 * (c + 1)],
            )
            nc.scalar.dma_start(
                out=wp4[:, H * c : H * (c + 1)],
                in_=wp4_d[128 * c : 128 * (c + 1), :],
            )

        a2ps = psump.tile([128, 512], F32, tag="proj")

        # --- PE warmup: ~2.5us of junk matmuls so HAM unthrottles early.
        # a2ps is safe garbage space: proj1's start=True overwrites it. ---
        for _ in range(6):
            nc.tensor.matmul(
                a2ps[:, :], lhsT=wct0[:, 0:128], rhs=wct0[:, 0:512],
                start=True, stop=True,
            )

        # --- layer 1: controller ---
        for m in range(NM):
            ms = slice(m * 128, (m + 1) * 128)
            for nh in range(2):
                c0, c1 = 2 * nh, 2 * nh + 1
                n1 = psum.tile([128, 1024], F32, tag="n1")
                for ch, c in ((0, c0), (1, c1)):
                    nc.tensor.matmul(
                        n1[:, 512 * ch : 512 * (ch + 1)],
                        lhsT=wct0[:, ms],
                        rhs=xc0[:, 512 * c : 512 * (c + 1)],
                        start=True,
                        stop=False,
                    )
                # K=64 passes for both chunks, concurrent in row groups 0/64
                nc.tensor.matmul(
                    n1[:, 0:512],
                    lhsT=wct1d[0:64, ms],
                    rhs=xc1d[0:64, 512 * c0 : 512 * (c0 + 1)],
                    start=False,
                    stop=True,
                    tile_position=(0, 0),
                )
                nc.tensor.matmul(
                    n1[:, 512:1024],
                    lhsT=wct1d[64:128, ms],
                    rhs=xc1d[64:128, 512 * c1 : 512 * (c1 + 1)],
                    start=False,
                    stop=True,
                    tile_position=(64, 0),
                )
                a1 = acts.tile([128, 1024], BF16, tag="a1")
                nc.scalar.activation(a1, n1, AF.Tanh, bias=cb18[:, m : m + 1])
                for ch, c in ((0, c0), (1, c1)):
                    nc.tensor.matmul(
                        a2ps[32 * c : 32 * c + 1, :],
                        lhsT=clw8[:, m : m + 1],
                        rhs=a1[:, 512 * ch : 512 * (ch + 1)],
                        start=(m == 0),
                        stop=(m == NM - 1),
                        tile_position=(0, 32 * c),
                    )

        # a2 chunk c -> partition 32c of the plant rhs (and to DRAM)
        for c in range(NC4):
            src = a2ps[32 * c : 32 * c + 1, :]
            dst = xp4[32 * c : 32 * c + 1, 512 * c : 512 * (c + 1)]
            if c < 2:
                nc.vector.tensor_copy(dst, src)
            else:
                nc.scalar.activation(dst, src, AF.Copy)
            nc.sync.dma_start(out=a2_d[0:1, 512 * c : 512 * (c + 1)], in_=dst)

        a4ps = psump.tile([128, 512], F32, tag="proj")
        a4sb = const.tile([128, 512], F32)

        # --- layer 2: plant ---
        for m in range(NM):
            for nh in range(2):
                c0, c1 = 2 * nh, 2 * nh + 1
                n3 = psum.tile([128, 1024], F32, tag="n1")
                for ch, c in ((0, c0), (1, c1)):
                    nc.tensor.matmul(
                        n3[:, 512 * ch : 512 * (ch + 1)],
                        lhsT=wp4[:, H * c + m * 128 : H * c + (m + 1) * 128],
                        rhs=xp4[:, 512 * c : 512 * (c + 1)],
                        start=True,
                        stop=True,
                    )
                a3 = acts.tile([128, 1024], BF16, tag="a1")
                nc.scalar.activation(a3, n3, AF.Tanh, bias=pb18[:, m : m + 1])
                for ch, c in ((0, c0), (1, c1)):
                    nc.tensor.matmul(
                        a4ps[32 * c : 32 * c + 1, :],
                        lhsT=plw8[:, m : m + 1],
                        rhs=a3[:, 512 * ch : 512 * (ch + 1)],
                        start=(m == 0),
                        stop=(m == NM - 1),
                        tile_position=(0, 32 * c),
                    )

        for c in range(NC4):
            src = a4ps[32 * c : 32 * c + 1, :]
            dst = a4sb[32 * c : 32 * c + 1, :]
            if c < 2:
                nc.vector.tensor_copy(dst, src)
            else:
                nc.scalar.activation(dst, src, AF.Copy)
            nc.sync.dma_start(out=a4_d[0:1, 512 * c : 512 * (c + 1)], in_=dst)

    nc.finalize()
    return nc


def _get_nc():
    if "nc" not in _CACHE:
        _CACHE["nc"] = _build_nc()
    return _CACHE["nc"]


def _roll_chunks(x):
    """Rotate each per-core 512-column chunk c by 32c along axis 0."""
    r, _ = x.shape
    v = x.reshape(r, N_CORES, NC4, 512)
    out = np.empty_like(v)
    for c in range(NC4):
        out[:, :, c, :] = np.roll(v[:, :, c, :], 32 * c, axis=0)
    return out.reshape(r, B)


def run(inputs, trace=False):
    """Run the device kernel. Returns (outputs_tuple, exec_time_ns|None)."""
    from concourse.bass_utils import run_bass_kernel_spmd

    f32 = np.float32
    g = {k: np.asarray(v, dtype=f32) for k, v in inputs.items()}

    # Host-side packing (tiny arrays; negligible cost vs device work).
    xc = np.ascontiguousarray(np.concatenate([g["citdl"], g["crtdl"]], axis=0))
    xc1d = np.ascontiguousarray(np.concatenate([g["cotdl"], g["cotdl"]], axis=0))
    # plant rhs, canonical K-order: [a2-slot; pitdl; potdl], then per-chunk roll
    xp_canon = np.concatenate(
        [np.zeros((1, B), f32), g["pitdl"], g["potdl"]], axis=0
    )
    xp4 = np.ascontiguousarray(_roll_chunks(xp_canon))

    wct0 = np.ascontiguousarray(
        np.concatenate([g["ciw"], g["crw"]], axis=1).T
    )  # [128, H]
    cowT = np.ascontiguousarray(g["cow"].T)  # [64, H]
    wct1d = np.ascontiguousarray(np.concatenate([cowT, cowT], axis=0))
    # plant weightsT, canonical K-order matching xp_canon
    wpt_canon = np.ascontiguousarray(
        np.concatenate([g["piw"][:, 63:64], g["piw"][:, :63], g["p_ow"]], axis=1).T
    )  # [128, H]
    wp4 = np.ascontiguousarray(
        np.concatenate(
            [np.roll(wpt_canon, 32 * c, axis=0) for c in range(NC4)], axis=0
        )
    )  # [512, H]

    import ml_dtypes

    bf16 = ml_dtypes.bfloat16
    clw8 = np.ascontiguousarray(g["clw"][0].reshape(NM, 128).T).astype(bf16)
    plw8 = np.ascontiguousarray(g["plw"][0].reshape(NM, 128).T).astype(bf16)
    cb18 = np.ascontiguousarray(g["cb1"][:, 0].reshape(NM, 128).T)
    pb1p = g["pb1"] + g["piw"][:, 63:64] * g["cb2"][0, 0]
    pb18 = np.ascontiguousarray(pb1p[:, 0].reshape(NM, 128).T)

    in_maps = []
    for c in range(N_CORES):
        cs = slice(c * BC, (c + 1) * BC)
        in_maps.append(
            {
                "xc": np.ascontiguousarray(xc[:, cs]),
                "xc1d": np.ascontiguousarray(xc1d[:, cs]),
                "xp4": np.ascontiguousarray(xp4[:, cs]),
                "wct0": wct0,
                "wct1d": wct1d,
                "wp4": wp4,
                "clw8": clw8,
                "plw8": plw8,
                "cb18": cb18,
                "pb18": pb18,
            }
        )

    nc = _get_nc()
    res = run_bass_kernel_spmd(nc, in_maps, list(range(N_CORES)), trace=trace)

    a2 = np.concatenate([res.results[c]["a2o"] for c in range(N_CORES)], axis=1)
    a4 = np.concatenate([res.results[c]["a4o"] for c in range(N_CORES)], axis=1)
    a2 = (a2 + g["cb2"][0, 0]).astype(f32)  # [1, B] controller output
    a4 = (a4 + g["pb2"][0, 0]).astype(f32)  # [1, B] plant output

    out = (
        a4,
        np.concatenate([a2, g["citdl"][:-1]], axis=0),
        np.concatenate([g["reference"], g["crtdl"][:-1]], axis=0),
        np.concatenate([a4, g["cotdl"][:-1]], axis=0),
        np.concatenate([a2, g["pitdl"][:-1]], axis=0),
        np.concatenate([a4, g["potdl"][:-1]], axis=0),
    )
    return out, res.exec_time_ns


def kernel(**inputs):
    return run(inputs, trace=False)[0]


# revision 9
# speedup vs baseline: 1.8124x; 1.2835x over previous
"""Trainium2 Bass kernel for nn_ControllerCell (controller+plant MLP cell).

Contract: kernel(**inputs) takes FULL unsharded inputs (numpy, float32) and
returns the FULL output tuple matching the reference:
    (a4, citdl', crtdl', cotdl', pitdl', potdl')

Strategy:
  - Data-parallel over the batch (column) dim B=16384 across 8 NeuronCores
    (2048 columns per core); tiny weight matrices replicated.
  - TDL shift updates are pure row shifts of the inputs -> assembled on the
    host. Only the computed rows (a2 = controller output, a4 = plant output)
    require device compute.
  - Device math per core (bias terms folded on host where possible):
        n1 = Wc @ [citdl;crtdl;cotdl] (+cb1 via ACT bias), a1 = tanh(n1)
        a2 = clw @ a1                  (cb2 added on host)
        n3 = Wp' @ perm[a2;pitdl;potdl] (+pb1' via ACT bias), a3 = tanh(n3)
        a4 = plw @ a3                  (pb2 added on host)
  - Projections (clw@a1, plw@a3) use 4-way PE column-tiling: column chunk c
    writes its [1,512] result to PSUM partition 32c, all four concurrently.
  - The plant rhs uses a per-chunk ROTATED K-order (rotate by 32c) so that
    chunk c's a2 row sits at partition 32c -- exactly where the projection
    left it (engine copies are partition-preserving). The plant weights are
    pre-rotated per chunk on the host to match.
  - MM1's K=192 runs as a K=128 pass plus K=64 passes packed pairwise into
    PE row groups 0-63 / 64-127 (cotdl + its weights duplicated into the
    upper partitions) so two column chunks proceed concurrently.
  - All matmuls in float32r (full-rate fp32 on the PE array).
"""

import numpy as np

N_CORES = 8
B = 16384
BC = B // N_CORES  # 2048 columns per core
H = 1024
NM = H // 128      # 8 M-tiles of 128 rows
NC4 = 4            # column chunks of 512 per core

_CACHE = {}


def _build_nc():
    import concourse.mybir as mybir
    import concourse.tile as tile
    from concourse import bacc
    from contextlib import ExitStack

    F32 = mybir.dt.float32
    F32R = mybir.dt.float32r
    BF16 = mybir.dt.bfloat16
    AF = mybir.ActivationFunctionType

    nc = bacc.Bacc()

    # xc: [citdl; crtdl] (128 rows). xc1d: cotdl duplicated (rows 0-63 and
    # 64-127 identical). xp4: per-chunk rotated [a2-slot; pitdl; potdl].
    # wp4: per-chunk rotated plant weightsT, stacked [4*128, H].
    xc_d = nc.declare_dram_parameter("xc", [128, BC], BF16, isOutput=False)
    xc1_d = nc.declare_dram_parameter("xc1d", [128, BC], BF16, isOutput=False)
    xp_d = nc.declare_dram_parameter("xp4", [128, BC], BF16, isOutput=False)
    wct0_d = nc.declare_dram_parameter("wct0", [128, H], BF16, isOutput=False)
    wct1_d = nc.declare_dram_parameter("wct1d", [128, H], BF16, isOutput=False)
    wp4_d = nc.declare_dram_parameter("wp4", [4 * 128, H], BF16, isOutput=False)
    clw_d = nc.declare_dram_parameter("clw8", [128, NM], BF16, isOutput=False)
    plw_d = nc.declare_dram_parameter("plw8", [128, NM], BF16, isOutput=False)
    cb1_d = nc.declare_dram_parameter("cb18", [128, NM], F32, isOutput=False)
    pb1_d = nc.declare_dram_parameter("pb18", [128, NM], F32, isOutput=False)
    a2_d = nc.declare_dram_parameter("a2o", [1, BC], BF16, isOutput=True)
    a4_d = nc.declare_dram_parameter("a4o", [1, BC], F32, isOutput=True)

    with ExitStack() as ctx:
        tc = ctx.enter_context(tile.TileContext(nc))
        const = ctx.enter_context(tc.tile_pool(name="const", bufs=1))
        acts = ctx.enter_context(tc.tile_pool(name="acts", bufs=3))
        psum = ctx.enter_context(tc.tile_pool(name="psum", bufs=2, space="PSUM"))
        psump = ctx.enter_context(tc.tile_pool(name="psump", bufs=1, space="PSUM"))

        # --- loads; sync + scalar drive the two HWDGE rings ---
        wct0 = const.tile([128, H], BF16)
        nc.sync.dma_start(out=wct0, in_=wct0_d[:, :])
        xc0 = const.tile([128, BC], BF16)
        xc1d = const.tile([128, BC], BF16)
        wct1d = const.tile([128, H], BF16)
        for c in (0, 1):
            cs = slice(512 * c, 512 * (c + 1))
            nc.sync.dma_start(out=xc0[:, cs], in_=xc_d[:, cs])
        nc.sync.dma_start(out=wct1d, in_=wct1_d[:, :])
        for c in (0, 1):
            cs = slice(512 * c, 512 * (c + 1))
            nc.sync.dma_start(out=xc1d[:, cs], in_=xc1_d[:, cs])
        for c in (2, 3):
            cs = slice(512 * c, 512 * (c + 1))
            nc.sync.dma_start(out=xc0[:, cs], in_=xc_d[:, cs])
            nc.sync.dma_start(out=xc1d[:, cs], in_=xc1_d[:, cs])

        clw8 = const.tile([128, NM], BF16)
        nc.scalar.dma_start(out=clw8, in_=clw_d[:, :])
        cb18 = const.tile([128, NM], F32)
        nc.scalar.dma_start(out=cb18, in_=cb1_d[:, :])
        plw8 = const.tile([128, NM], BF16)
        nc.scalar.dma_start(out=plw8, in_=plw_d[:, :])
        pb18 = const.tile([128, NM], F32)
        nc.scalar.dma_start(out=pb18, in_=pb1_d[:, :])
        xp4 = const.tile([128, BC], BF16)
        wp4 = const.tile([128, 4 * H], BF16)
        for c in range(NC4):
            nc.scalar.dma_start(
                out=xp4[:, 512 * c : 512 * (c + 1)],
                in_=xp_d[:, 512 * c : 512 * (c + 1)],
            )
            nc.scalar.dma_start(
                out=wp4[:, H * c : H * (c + 1)],
                in_=wp4_d[128 * c : 128 * (c + 1), :],
            )

        a2ps = psump.tile([128, 512], F32, tag="proj")

        # --- PE warmup: ~2.5us of junk matmuls so HAM unthrottles early.
        # a2ps is safe garbage space: proj1's start=True overwrites it. ---
        for _ in range(10):
            nc.tensor.matmul(
                a2ps[:, :], lhsT=wct0[:, 0:128], rhs=wct0[:, 0:512],
                start=True, stop=True,
            )

        # --- layer 1: controller ---
        for m in range(NM):
            ms = slice(m * 128, (m + 1) * 128)
            for nh in range(2):
                c0, c1 = 2 * nh, 2 * nh + 1
                n1 = psum.tile([128, 1024], F32, tag="n1")
                for ch, c in ((0, c0), (1, c1)):
                    nc.tensor.matmul(
                        n1[:, 512 * ch : 512 * (ch + 1)],
                        lhsT=wct0[:, ms],
                        rhs=xc0[:, 512 * c : 512 * (c + 1)],
                        start=True,
                        stop=False,
                    )
                # K=64 passes for both chunks, concurrent in row groups 0/64
                nc.tensor.matmul(
                    n1[:, 0:512],
                    lhsT=wct1d[0:64, ms],
                    rhs=xc1d[0:64, 512 * c0 : 512 * (c0 + 1)],
                    start=False,
                    stop=True,
                    tile_position=(0, 0),
                )
                nc.tensor.matmul(
                    n1[:, 512:1024],
                    lhsT=wct1d[64:128, ms],
                    rhs=xc1d[64:128, 512 * c1 : 512 * (c1 + 1)],
                    start=False,
                    stop=True,
                    tile_position=(64, 0),
                )
                a1 = acts.tile([128, 1024], BF16, tag="a1")
                nc.scalar.activation(a1, n1, AF.Tanh, bias=cb18[:, m : m + 1])
                for ch, c in ((0, c0), (1, c1)):
                    nc.tensor.matmul(
                        a2ps[32 * c : 32 * c + 1, :],
                        lhsT=clw8[:, m : m + 1],
                        rhs=a1[:, 512 * ch : 512 * (ch + 1)],
                        start=(m == 0),
                        stop=(m == NM - 1),
                        tile_position=(0, 32 * c),
                    )

        # a2 chunk c -> partition 32c of the plant rhs (and to DRAM)
        for c in range(NC4):
            src = a2ps[32 * c : 32 * c + 1, :]
            dst = xp4[32 * c : 32 * c + 1, 512 * c : 512 * (c + 1)]
            if c < 2:
                nc.vector.tensor_copy(dst, src)
            else:
                nc.scalar.activation(dst, src, AF.Copy)
            nc.sync.dma_start(out=a2_d[0:1, 512 * c : 512 * (c + 1)], in_=dst)

        a4ps = psump.tile([128, 512], F32, tag="proj")
        a4sb = const.tile([128, 512], F32)

        # --- layer 2: plant ---
        for m in range(NM):
            for nh in range(2):
                c0, c1 = 2 * nh, 2 * nh + 1
                n3 = psum.tile([128, 1024], F32, tag="n1")
                for ch, c in ((0, c0), (1, c1)):
                    nc.tensor.matmul(
                        n3[:, 512 * ch : 512 * (ch + 1)],
                        lhsT=wp4[:, H * c + m * 128 : H * c + (m + 1) * 128],
                        rhs=xp4[:, 512 * c : 512 * (c + 1)],
                        start=True,
                        stop=True,
                    )
                a3 = acts.tile([128, 1024], BF16, tag="a1")
                nc.scalar.activation(a3, n3, AF.Tanh, bias=pb18[:, m : m + 1])
                for ch, c in ((0, c0), (1, c1)):
                    nc.tensor.matmul(
                        a4ps[32 * c : 32 * c + 1, :],
                        lhsT=plw8[:, m : m + 1],
                        rhs=a3[:, 512 * ch : 512 * (ch + 1)],
                        start=(m == 0),
                        stop=(m == NM - 1),
                        tile_position=(0, 32 * c),
                    )

        for c in range(NC4):
            src = a4ps[32 * c : 32 * c + 1, :]
            dst = a4sb[32 * c : 32 * c + 1, :]
            if c < 2:
                nc.vector.tensor_copy(dst, src)
            else:
                nc.scalar.activation(dst, src, AF.Copy)
            nc.sync.dma_start(out=a4_d[0:1, 512 * c : 512 * (c + 1)], in_=dst)

    nc.finalize()
    return nc


def _get_nc():
    if "nc" not in _CACHE:
        _CACHE["nc"] = _build_nc()
    return _CACHE["nc"]


def _roll_chunks(x):
    """Rotate each per-core 512-column chunk c by 32c along axis 0."""
    r, _ = x.shape
    v = x.reshape(r, N_CORES, NC4, 512)
    out = np.empty_like(v)
    for c in range(NC4):
        out[:, :, c, :] = np.roll(v[:, :, c, :], 32 * c, axis=0)
    return out.reshape(r, B)


def run(inputs, trace=False):
    """Run the device kernel. Returns (outputs_tuple, exec_time_ns|None)."""
    from concourse.bass_utils import run_bass_kernel_spmd

    f32 = np.float32
    g = {k: np.asarray(v, dtype=f32) for k, v in inputs.items()}

    # Host-side packing (tiny arrays; negligible cost vs device work).
    xc = np.ascontiguousarray(np.concatenate([g["citdl"], g["crtdl"]], axis=0))
    xc1d = np.ascontiguousarray(np.concatenate([g["cotdl"], g["cotdl"]], axis=0))
    # plant rhs, canonical K-order: [a2-slot; pitdl; potdl], then per-chunk roll
    xp_canon = np.concatenate(
        [np.zeros((1, B), f32), g["pitdl"], g["potdl"]], axis=0
    )
    xp4 = np.ascontiguousarray(_roll_chunks(xp_canon))

    wct0 = np.ascontiguousarray(
        np.concatenate([g["ciw"], g["crw"]], axis=1).T
    )  # [128, H]
    cowT = np.ascontiguousarray(g["cow"].T)  # [64, H]
    wct1d = np.ascontiguousarray(np.concatenate([cowT, cowT], axis=0))
    # plant weightsT, canonical K-order matching xp_canon
    wpt_canon = np.ascontiguousarray(
        np.concatenate([g["piw"][:, 63:64], g["piw"][:, :63], g["p_ow"]], axis=1).T
    )  # [128, H]
    wp4 = np.ascontiguousarray(
        np.concatenate(
            [np.roll(wpt_canon, 32 * c, axis=0) for c in range(NC4)], axis=0
        )
    )  # [512, H]

    import ml_dtypes

    bf16 = ml_dtypes.bfloat16
    clw8 = np.ascontiguousarray(g["clw"][0].reshape(NM, 128).T).astype(bf16)
    plw8 = np.ascontiguousarray(g["plw"][0].reshape(NM, 128).T).astype(bf16)
    xc = xc.astype(bf16)
    xc1d = xc1d.astype(bf16)
    xp4 = xp4.astype(bf16)
    wct0 = wct0.astype(bf16)
    wct1d = wct1d.astype(bf16)
    wp4 = wp4.astype(bf16)
    cb18 = np.ascontiguousarray(g["cb1"][:, 0].reshape(NM, 128).T)
    pb1p = g["pb1"] + g["piw"][:, 63:64] * g["cb2"][0, 0]
    pb18 = np.ascontiguousarray(pb1p[:, 0].reshape(NM, 128).T)

    in_maps = []
    for c in range(N_CORES):
        cs = slice(c * BC, (c + 1) * BC)
        in_maps.append(
            {
                "xc": np.ascontiguousarray(xc[:, cs]),
                "xc1d": np.ascontiguousarray(xc1d[:, cs]),
                "xp4": np.ascontiguousarray(xp4[:, cs]),
                "wct0": wct0,
                "wct1d": wct1d,
                "wp4": wp4,
                "clw8": clw8,
                "plw8": plw8,
                "cb18": cb18,
                "pb18": pb18,
            }
        )

    nc = _get_nc()
    res = run_bass_kernel_spmd(nc, in_maps, list(range(N_CORES)), trace=trace)

    a2 = np.concatenate([res.results[c]["a2o"] for c in range(N_CORES)], axis=1)
    a4 = np.concatenate([res.results[c]["a4o"] for c in range(N_CORES)], axis=1)
    a2 = (a2.astype(f32) + g["cb2"][0, 0]).astype(f32)  # [1, B] controller output
    a4 = (a4 + g["pb2"][0, 0]).astype(f32)  # [1, B] plant output

    out = (
        a4,
        np.concatenate([a2, g["citdl"][:-1]], axis=0),
        np.concatenate([g["reference"], g["crtdl"][:-1]], axis=0),
        np.concatenate([a4, g["cotdl"][:-1]], axis=0),
        np.concatenate([a2, g["pitdl"][:-1]], axis=0),
        np.concatenate([a4, g["potdl"][:-1]], axis=0),
    )
    return out, res.exec_time_ns


def kernel(**inputs):
    return run(inputs, trace=False)[0]


# revision 10
# speedup vs baseline: 2.1829x; 1.2044x over previous
"""Trainium2 Bass kernel for nn_ControllerCell (controller+plant MLP cell).

Contract: kernel(**inputs) takes FULL unsharded inputs (numpy, float32) and
returns the FULL output tuple matching the reference:
    (a4, citdl', crtdl', cotdl', pitdl', potdl')

Strategy:
  - Data-parallel over the batch (column) dim B=16384 across 8 NeuronCores
    (2048 columns per core); tiny weight matrices replicated.
  - TDL shift updates are pure row shifts of the inputs -> assembled on the
    host. Only the computed rows (a2 = controller output, a4 = plant output)
    require device compute.
  - Device math per core (bias terms folded on host where possible):
        n1 = Wc @ [citdl;crtdl;cotdl] (+cb1 via ACT bias), a1 = tanh(n1)
        a2 = clw @ a1                  (cb2 added on host)
        n3 = Wp' @ perm[a2;pitdl;potdl] (+pb1' via ACT bias), a3 = tanh(n3)
        a4 = plw @ a3                  (pb2 added on host)
  - Projections (clw@a1, plw@a3) use 4-way PE column-tiling: column chunk c
    writes its [1,512] result to PSUM partition 32c, all four concurrently.
  - The plant rhs uses a per-chunk ROTATED K-order (rotate by 32c) so that
    chunk c's a2 row sits at partition 32c -- exactly where the projection
    left it (engine copies are partition-preserving). The plant weights are
    pre-rotated per chunk on the host to match.
  - MM1's K=192 runs as a K=128 pass plus K=64 passes packed pairwise into
    PE row groups 0-63 / 64-127 (cotdl + its weights duplicated into the
    upper partitions) so two column chunks proceed concurrently.
  - All matmuls in float32r (full-rate fp32 on the PE array).
"""

import numpy as np

N_CORES = 8
B = 16384
BC = B // N_CORES  # 2048 columns per core
H = 1024
NM = H // 128      # 8 M-tiles of 128 rows
NC4 = 4            # column chunks of 512 per core

_CACHE = {}


def _build_nc():
    import concourse.mybir as mybir
    import concourse.tile as tile
    from concourse import bacc
    from contextlib import ExitStack

    F32 = mybir.dt.float32
    F32R = mybir.dt.float32r
    BF16 = mybir.dt.bfloat16
    AF = mybir.ActivationFunctionType

    nc = bacc.Bacc()

    # xc: [citdl; crtdl] (128 rows). xc1d: cotdl duplicated (rows 0-63 and
    # 64-127 identical). xp4: per-chunk rotated [a2-slot; pitdl; potdl].
    # wp4: per-chunk rotated plant weightsT, stacked [4*128, H].
    xc_d = nc.declare_dram_parameter("xc", [128, BC], BF16, isOutput=False)
    xc1_d = nc.declare_dram_parameter("xc1d", [128, BC], BF16, isOutput=False)
    xp_d = nc.declare_dram_parameter("xp4", [128, BC], BF16, isOutput=False)
    wct0_d = nc.declare_dram_parameter("wct0", [128, H], BF16, isOutput=False)
    wct1_d = nc.declare_dram_parameter("wct1d", [128, H], BF16, isOutput=False)
    wp4_d = nc.declare_dram_parameter("wp4", [4 * 128, H], BF16, isOutput=False)
    clw_d = nc.declare_dram_parameter("clw8", [128, NM], BF16, isOutput=False)
    plw_d = nc.declare_dram_parameter("plw8", [128, NM], BF16, isOutput=False)
    cb1_d = nc.declare_dram_parameter("cb18", [128, NM], F32, isOutput=False)
    pb1_d = nc.declare_dram_parameter("pb18", [128, NM], F32, isOutput=False)
    a2_d = nc.declare_dram_parameter("a2o", [1, BC], BF16, isOutput=True)
    a4_d = nc.declare_dram_parameter("a4o", [1, BC], F32, isOutput=True)

    with ExitStack() as ctx:
        tc = ctx.enter_context(tile.TileContext(nc))
        const = ctx.enter_context(tc.tile_pool(name="const", bufs=1))
        acts = ctx.enter_context(tc.tile_pool(name="acts", bufs=3))
        psum = ctx.enter_context(tc.tile_pool(name="psum", bufs=3, space="PSUM"))
        psump = ctx.enter_context(tc.tile_pool(name="psump", bufs=2, space="PSUM"))

        # --- loads; sync + scalar drive the two HWDGE rings, first-half-first ---
        wct0 = const.tile([128, H], BF16)
        nc.sync.dma_start(out=wct0, in_=wct0_d[:, :])
        xc0 = const.tile([128, BC], BF16)
        xc1d = const.tile([128, BC], BF16)
        wct1d = const.tile([128, H], BF16)
        nc.sync.dma_start(out=xc0[:, 0:1024], in_=xc_d[:, 0:1024])
        nc.sync.dma_start(out=xc1d[:, 0:1024], in_=xc1_d[:, 0:1024])
        nc.sync.dma_start(out=wct1d, in_=wct1_d[:, :])
        nc.sync.dma_start(out=xc0[:, 1024:2048], in_=xc_d[:, 1024:2048])
        nc.sync.dma_start(out=xc1d[:, 1024:2048], in_=xc1_d[:, 1024:2048])

        clw8 = const.tile([128, NM], BF16)
        nc.scalar.dma_start(out=clw8, in_=clw_d[:, :])
        cb18 = const.tile([128, NM], F32)
        nc.scalar.dma_start(out=cb18, in_=cb1_d[:, :])
        plw8 = const.tile([128, NM], BF16)
        nc.scalar.dma_start(out=plw8, in_=plw_d[:, :])
        pb18 = const.tile([128, NM], F32)
        nc.scalar.dma_start(out=pb18, in_=pb1_d[:, :])
        xp4 = const.tile([128, BC], BF16)
        wp4 = const.tile([128, 4 * H], BF16)
        for c in range(NC4):
            nc.scalar.dma_start(
                out=xp4[:, 512 * c : 512 * (c + 1)],
                in_=xp_d[:, 512 * c : 512 * (c + 1)],
            )
            nc.scalar.dma_start(
                out=wp4[:, H * c : H * (c + 1)],
                in_=wp4_d[128 * c : 128 * (c + 1), :],
            )

        a2ps = psump.tile([128, 512], F32, tag="proj")
        a4ps = psump.tile([128, 512], F32, tag="proj")
        a4sb = const.tile([128, 512], F32)

        # --- PE warmup from a memset tile (no DMA dependency): dense junk
        # matmuls so HAM unthrottles before real work. a2ps is safe garbage
        # space: proj1's start=True overwrites its region. ---
        warm = const.tile([128, 512], BF16)
        nc.vector.memset(warm, 0.25)
        for _ in range(8):
            nc.tensor.matmul(
                a2ps[:, :], lhsT=warm[:, 0:128], rhs=warm[:, :],
                start=True, stop=True,
            )

        def layer1(h):
            cA, cB = 2 * h, 2 * h + 1
            sA = slice(512 * cA, 512 * (cA + 1))
            sB = slice(512 * cB, 512 * (cB + 1))
            for m in range(NM):
                ms = slice(m * 128, (m + 1) * 128)
                n1 = psum.tile([128, 1024], F32, tag="n1", name=f"n1_{h}_{m}")
                nc.tensor.matmul(n1[:, 0:512], lhsT=wct0[:, ms], rhs=xc0[:, sA],
                                 start=True, stop=False)
                nc.tensor.matmul(n1[:, 512:1024], lhsT=wct0[:, ms], rhs=xc0[:, sB],
                                 start=True, stop=False)
                nc.tensor.matmul(n1[:, 0:512], lhsT=wct1d[0:64, ms],
                                 rhs=xc1d[0:64, sA], start=False, stop=True,
                                 tile_position=(0, 0))
                nc.tensor.matmul(n1[:, 512:1024], lhsT=wct1d[64:128, ms],
                                 rhs=xc1d[64:128, sB], start=False, stop=True,
                                 tile_position=(64, 0))
                a1 = acts.tile([128, 1024], BF16, tag="a1", name=f"a1_{h}_{m}")
                nc.scalar.activation(a1, n1, AF.Tanh, bias=cb18[:, m : m + 1])
                for ch, c in ((0, cA), (1, cB)):
                    nc.tensor.matmul(
                        a2ps[32 * c : 32 * c + 1, :],
                        lhsT=clw8[:, m : m + 1],
                        rhs=a1[:, 512 * ch : 512 * (ch + 1)],
                        start=(m == 0),
                        stop=(m == NM - 1),
                        tile_position=(0, 32 * c),
                    )

        def a2_copies(h):
            for c in (2 * h, 2 * h + 1):
                src_ = a2ps[32 * c : 32 * c + 1, :]
                dst = xp4[32 * c : 32 * c + 1, 512 * c : 512 * (c + 1)]
                nc.vector.tensor_copy(dst, src_)
                nc.sync.dma_start(out=a2_d[0:1, 512 * c : 512 * (c + 1)], in_=dst)

        def layer2(h):
            cA, cB = 2 * h, 2 * h + 1
            sA = slice(512 * cA, 512 * (cA + 1))
            sB = slice(512 * cB, 512 * (cB + 1))
            for m in range(NM):
                n3 = psum.tile([128, 1024], F32, tag="n1", name=f"n3_{h}_{m}")
                nc.tensor.matmul(
                    n3[:, 0:512],
                    lhsT=wp4[:, H * cA + m * 128 : H * cA + (m + 1) * 128],
                    rhs=xp4[:, sA], start=True, stop=True)
                nc.tensor.matmul(
                    n3[:, 512:1024],
                    lhsT=wp4[:, H * cB + m * 128 : H * cB + (m + 1) * 128],
                    rhs=xp4[:, sB], start=True, stop=True)
                a3 = acts.tile([128, 1024], BF16, tag="a1", name=f"a3_{h}_{m}")
                nc.scalar.activation(a3, n3, AF.Tanh, bias=pb18[:, m : m + 1])
                for ch, c in ((0, cA), (1, cB)):
                    nc.tensor.matmul(
                        a4ps[32 * c : 32 * c + 1, :],
                        lhsT=plw8[:, m : m + 1],
                        rhs=a3[:, 512 * ch : 512 * (ch + 1)],
                        start=(m == 0),
                        stop=(m == NM - 1),
                        tile_position=(0, 32 * c),
                    )

        def a4_copies(h):
            for c in (2 * h, 2 * h + 1):
                src_ = a4ps[32 * c : 32 * c + 1, :]
                dst = a4sb[32 * c : 32 * c + 1, :]
                nc.vector.tensor_copy(dst, src_)
                nc.sync.dma_start(out=a4_d[0:1, 512 * c : 512 * (c + 1)], in_=dst)

        layer1(0)
        layer1(1)
        a2_copies(0)
        layer2(0)
        a2_copies(1)
        layer2(1)
        a4_copies(0)
        a4_copies(1)

    nc.finalize()
    return nc


def _get_nc():
    if "nc" not in _CACHE:
        _CACHE["nc"] = _build_nc()
    return _CACHE["nc"]


def _roll_chunks(x):
    """Rotate each per-core 512-column chunk c by 32c along axis 0."""
    r, _ = x.shape
    v = x.reshape(r, N_CORES, NC4, 512)
    out = np.empty_like(v)
    for c in range(NC4):
        out[:, :, c, :] = np.roll(v[:, :, c, :], 32 * c, axis=0)
    return out.reshape(r, B)


def run(inputs, trace=False):
    """Run the device kernel. Returns (outputs_tuple, exec_time_ns|None)."""
    from concourse.bass_utils import run_bass_kernel_spmd

    f32 = np.float32
    g = {k: np.asarray(v, dtype=f32) for k, v in inputs.items()}

    # Host-side packing (tiny arrays; negligible cost vs device work).
    xc = np.ascontiguousarray(np.concatenate([g["citdl"], g["crtdl"]], axis=0))
    xc1d = np.ascontiguousarray(np.concatenate([g["cotdl"], g["cotdl"]], axis=0))
    # plant rhs, canonical K-order: [a2-slot; pitdl; potdl], then per-chunk roll
    xp_canon = np.concatenate(
        [np.zeros((1, B), f32), g["pitdl"], g["potdl"]], axis=0
    )
    xp4 = np.ascontiguousarray(_roll_chunks(xp_canon))

    wct0 = np.ascontiguousarray(
        np.concatenate([g["ciw"], g["crw"]], axis=1).T
    )  # [128, H]
    cowT = np.ascontiguousarray(g["cow"].T)  # [64, H]
    wct1d = np.ascontiguousarray(np.concatenate([cowT, cowT], axis=0))
    # plant weightsT, canonical K-order matching xp_canon
    wpt_canon = np.ascontiguousarray(
        np.concatenate([g["piw"][:, 63:64], g["piw"][:, :63], g["p_ow"]], axis=1).T
    )  # [128, H]
    wp4 = np.ascontiguousarray(
        np.concatenate(
            [np.roll(wpt_canon, 32 * c, axis=0) for c in range(NC4)], axis=0
        )
    )  # [512, H]

    import ml_dtypes

    bf16 = ml_dtypes.bfloat16
    clw8 = np.ascontiguousarray(g["clw"][0].reshape(NM, 128).T).astype(bf16)
    plw8 = np.ascontiguousarray(g["plw"][0].reshape(NM, 128).T).astype(bf16)
    xc = xc.astype(bf16)
    xc1d = xc1d.astype(bf16)
    xp4 = xp4.astype(bf16)
    wct0 = wct0.astype(bf16)
    wct1d = wct1d.astype(bf16)
    wp4 = wp4.astype(bf16)
    cb18 = np.ascontiguousarray(g["cb1"][:, 0].reshape(NM, 128).T)
    pb1p = g["pb1"] + g["piw"][:, 63:64] * g["cb2"][0, 0]
    pb18 = np.ascontiguousarray(pb1p[:, 0].reshape(NM, 128).T)

    in_maps = []
    for c in range(N_CORES):
        cs = slice(c * BC, (c + 1) * BC)
        in_maps.append(
            {
                "xc": np.ascontiguousarray(xc[:, cs]),
                "xc1d": np.ascontiguousarray(xc1d[:, cs]),
                "xp4": np.ascontiguousarray(xp4[:, cs]),
                "wct0": wct0,
                "wct1d": wct1d,
                "wp4": wp4,
                "clw8": clw8,
                "plw8": plw8,
                "cb18": cb18,
                "pb18": pb18,
            }
        )

    nc = _get_nc()
    res = run_bass_kernel_spmd(nc, in_maps, list(range(N_CORES)), trace=trace)

    a2 = np.concatenate([res.results[c]["a2o"] for c in range(N_CORES)], axis=1)
    a4 = np.concatenate([res.results[c]["a4o"] for c in range(N_CORES)], axis=1)
    a2 = (a2.astype(f32) + g["cb2"][0, 0]).astype(f32)  # [1, B] controller output
    a4 = (a4 + g["pb2"][0, 0]).astype(f32)  # [1, B] plant output

    out = (
        a4,
        np.concatenate([a2, g["citdl"][:-1]], axis=0),
        np.concatenate([g["reference"], g["crtdl"][:-1]], axis=0),
        np.concatenate([a4, g["cotdl"][:-1]], axis=0),
        np.concatenate([a2, g["pitdl"][:-1]], axis=0),
        np.concatenate([a4, g["potdl"][:-1]], axis=0),
    )
    return out, res.exec_time_ns


def kernel(**inputs):
    return run(inputs, trace=False)[0]
